# revision 82
# baseline (speedup 1.0000x reference)
"""Trainium2 Bass kernel for nn_AttentionLayer (masked attention pooling).

Reference math (per batch row b):
    pq      = tanh(qe @ Wq.T + bq).reshape(D, H)
    dotted  = item_b @ pq                  # (N, H)
    reduced = dotted @ Wr.T                # (N, 1)
    ... masked softmax over N (max-subtracted, denom<1e-7 -> denom+1) ...
    out_b   = sum_n weight[n] * item_b[n]  # (D,)

Key algebraic simplification: reduced = item_b @ (pq @ Wr.T), so the H
dimension collapses into a per-row vector v_b = pq @ Wr.T of size D before
ever touching item_embedding.  Per row the work is then:
    scores = item_b @ v_b; masked softmax; out = weights @ item_b.

Distribution: pure data-parallel over batch across 8 cores (256 rows each).
Layout: batch rows on SBUF partitions (128 per tile), so every op is
per-partition independent -> VectorE/ScalarE; TensorE only does the shared
query projection qe @ Wq.T.

Softmax without max-subtraction: |score| <= ~50 so exp() stays within f32
range; softmax is shift-invariant so results match.  The reference's
`denom < 1e-7` test happens on max-subtracted values, reproduced here as
denom_raw < exp(smax + ln(1e-7)), and the +1 branch becomes
denom_raw + exp(smax).
"""

import sys

if "/opt/trn_rl_repo" not in sys.path:
    sys.path.insert(0, "/opt/trn_rl_repo")

from contextlib import ExitStack

import numpy as np

import concourse.bass as bass
import concourse.bacc as bacc
import concourse.tile as tile
from concourse import mybir

B, N, D, H = 2048, 200, 128, 8
NCORES = 8
BS = B // NCORES          # 256 batch rows per core
P = 128                   # partitions
NT = BS // P              # 2 batch tiles per core
C = 50                    # n-chunk size
NCH = N // C              # 4 chunks per batch tile
DH = D * H                # 1024

F32 = mybir.dt.float32
BF16 = mybir.dt.bfloat16
U8 = mybir.dt.uint8
AX = mybir.AxisListType
OP = mybir.AluOpType
ACT = mybir.ActivationFunctionType

_CACHE = {}


def build_module() -> bass.Bass:
    nc = bacc.Bacc("TRN2", target_bir_lowering=False)

    item = nc.declare_dram_parameter("item", [BS, N, D], F32, isOutput=False)
    mask = nc.declare_dram_parameter("mask", [BS, N], U8, isOutput=False)
    bq = nc.declare_dram_parameter("bq", [1, DH], BF16, isOutput=False)
    wr_rep_in = nc.declare_dram_parameter("Wr_rep", [1, DH], BF16, isOutput=False)
    wqT_in = nc.declare_dram_parameter("WqT", [D, DH], BF16, isOutput=False)
    qeT_in = nc.declare_dram_parameter("qeT", [D, BS], BF16, isOutput=False)
    out = nc.declare_dram_parameter("out", [BS, D], F32, isOutput=True)

    with tile.TileContext(nc) as tc, ExitStack() as ctx:
        const = ctx.enter_context(tc.tile_pool(name="const", bufs=1))
        psum = ctx.enter_context(tc.tile_pool(name="psum", bufs=2, space="PSUM"))
        work = ctx.enter_context(tc.tile_pool(name="work", bufs=2))
        items = ctx.enter_context(tc.tile_pool(name="items", bufs=2))
        itbs = ctx.enter_context(tc.tile_pool(name="itbs", bufs=4))
        tmps = ctx.enter_context(tc.tile_pool(name="tmps", bufs=3))
        small = ctx.enter_context(tc.tile_pool(name="small", bufs=4))

        # DMA issue order follows the dependency-chain lengths: wqT/qeT gate
        # the long PE->tanh->v chain, then the tiny bq/wr_rep transfers
        # (queues are FIFO — 2KB stuck behind an 800KB item quarter arrives
        # 8us late), then the first item chunk that gates the DVE stream.
        wqT = const.tile([P, DH], BF16)
        nc.sync.dma_start(wqT[:], wqT_in[:])
        qeT_all = const.tile([P, BS], BF16)
        nc.sync.dma_start(qeT_all[:], qeT_in[:])
        bq_sb = const.tile([1, DH], BF16)
        nc.sync.dma_start(bq_sb[:], bq[:])

        # Broadcast Wr (host-replicated to [1, D*H]) to all partitions with a
        # partition-stride-0 DMA read; the dense layout keeps the v-multiply
        # at full DVE rate (inner-broadcast APs run at a pathological rate).
        wr_rep = const.tile([P, DH], BF16)
        nc.sync.dma_start(wr_rep[:], wr_rep_in[0:1, :].to_broadcast([P, DH]))

        it0 = items.tile([P, C * D], F32, tag="item")
        src0 = item[0:P, 0:C, :].rearrange("b n d -> b (n d)")
        step0 = C * D // 4
        for q in range(4):
            nc.sync.dma_start(
                it0[:, q * step0:(q + 1) * step0], src0[:, q * step0:(q + 1) * step0])

        ones1 = const.tile([1, P], BF16)
        nc.vector.memset(ones1[:], 1.0)

        # ---- phase A for ALL tiles first, so ACT's static schedule finishes
        # the projections before starting the item converts that gate DVE.
        tiles = []
        for t in range(NT):
            rows = slice(t * P, (t + 1) * P)

            # query projection: v[b, d] = sum_h tanh(qe@WqT + bq) * Wr
            # (bf16 throughout: v is quantized to bf16 downstream anyway)
            pqt = work.tile([P, DH], BF16, tag="pqt")
            for j in range(2):
                js = slice(j * 512, (j + 1) * 512)
                pq_ps = psum.tile([P, 512], F32, tag="pq")
                nc.tensor.matmul(
                    pq_ps[:], qeT_all[:, rows], wqT[:, js], start=True, stop=False)
                nc.tensor.matmul(pq_ps[:], ones1[:], bq_sb[:, js], start=False, stop=True)
                nc.scalar.activation(pqt[:, js], pq_ps[:], ACT.Tanh)

            tmpv = work.tile([P, DH], BF16, tag="tmpv")
            tmpv3 = tmpv[:].rearrange("p (d h) -> p d h", h=H)
            nc.vector.tensor_tensor(tmpv[:], pqt[:], wr_rep[:], OP.mult)
            v_sb = work.tile([P, D], F32, tag="v")
            nc.vector.tensor_reduce(v_sb[:], tmpv3, axis=AX.X, op=OP.add)

            # bf16 working copies (all heavy DVE ops run bf16 at 2x)
            vb = work.tile([P, D], BF16, tag="vb")
            nc.vector.tensor_copy(out=vb[:], in_=v_sb[:])

            s_all = work.tile([P, N], F32, tag="s")
            # att is stored pair-duplicated ([a0,a0,a1,a1,...]) — written once
            # per chunk by a single fused multiply, consumed both by the
            # pair-packed wsum multiply and (summed, halved) by the denom
            att_dup = work.tile([P, 2 * N], BF16, tag="attd")
            acc = work.tile([P, D], F32, tag="acc")
            nc.gpsimd.memset(acc[:], 0.0)
            tiles.append((vb, s_all, att_dup, acc))

        for t in range(NT):
            rows = slice(t * P, (t + 1) * P)
            vb, s_all, att_dup, acc = tiles[t]

            # mask load sits here so its DMA descriptors don't delay the
            # first item chunks in the preamble queue (mask is first needed
            # only by the att multiply)
            mk_u8 = work.tile([P, N], U8, tag="mku")
            nc.sync.dma_start(mk_u8[:], mask[rows, :])
            mk_b = work.tile([P, N], BF16, tag="mkb")
            nc.vector.tensor_copy(out=mk_b[:], in_=mk_u8[:])

            # chunk list: (n-offset, width, f32 source AP or None).  Tile 0
            # splits its first chunk in two so compute starts as soon as the
            # first half of it0 lands.
            if t == 0:
                chunks = [(0, C // 2, it0[:, 0:C * D // 2]),
                          (C // 2, C // 2, it0[:, C * D // 2:C * D])]
                chunks += [(k * C, C, None) for k in range(1, NCH)]
            else:
                chunks = [(k * C, C, None) for k in range(NCH)]

            for ci, (off, w, it_src) in enumerate(chunks):
                ks = slice(off, off + w)
                if it_src is None:
                    it = items.tile([P, C * D], F32, tag="item")
                    # split the chunk DMA in halves to spread across queues
                    src = item[rows, ks, :].rearrange("b n d -> b (n d)")
                    hw = w * D // 2
                    nc.sync.dma_start(it[:, :hw], src[:, :hw])
                    nc.sync.dma_start(it[:, hw:w * D], src[:, hw:])
                    it_src = it[:, 0:w * D]

                # f32 -> bf16 convert on ACT (GpSimd CAST is ~4x slower and its
                # SBUF traffic stalls concurrent DVE ops).  The first chunks
                # convert on the still-idle DVE to shorten the ramp.
                itb = itbs.tile([P, C * D], BF16, tag="itb")
                if t == 0 and ci < 2:
                    nc.vector.tensor_copy(out=itb[:, 0:w * D], in_=it_src)
                else:
                    nc.scalar.copy(itb[:, 0:w * D], it_src)
                it3 = itb[:, 0:w * D].rearrange("p (n d) -> p n d", d=D)

                # scores chunk: s[p, n] = sum_d item * v
                # bf16 mult, dense halvings over d, then f32 reduce of 8
                tmp = tmps.tile([P, C * D], BF16, tag="tmp")
                tmp3 = tmp[:, 0:w * D].rearrange("p (n d) -> p n d", d=D)
                vb3 = vb[:, None, :].to_broadcast([P, w, D])
                nc.vector.tensor_tensor(tmp3, it3, vb3, OP.mult)
                dd = D
                while dd > 8:
                    dd //= 2
                    nc.vector.tensor_tensor(
                        tmp3[:, :, 0:dd], tmp3[:, :, 0:dd], tmp3[:, :, dd:2 * dd],
                        OP.add)
                nc.vector.tensor_reduce(
                    s_all[:, ks], tmp3[:, :, 0:8], axis=AX.X, op=OP.add)

                # att chunk: exp(s) * mask  (no max subtraction needed in f32)
                # att + pair-duplicate on DVE: tiny ops, and keeping them here
                # avoids two cross-engine semaphore hops in the chunk chain
                e_c = small.tile([P, C], BF16, tag="ec")
                nc.scalar.activation(e_c[:, 0:w], s_all[:, ks], ACT.Exp)
                # single fused op: compute e*mask directly into adjacent-pair
                # duplicated form (pairs pack into 32-bit reads -> 2x wmult)
                ad2 = att_dup[:, 2 * off:2 * (off + w)].rearrange(
                    "p (n two) -> p n two", two=2)
                nc.vector.tensor_tensor(
                    ad2,
                    e_c[:, 0:w][:, :, None].to_broadcast([P, w, 2]),
                    mk_b[:, ks][:, :, None].to_broadcast([P, w, 2]), OP.mult)

                # weighted accumulation: acc[p, d] += sum_n att * item
                # bf16 mult then dense pairwise tree over n (all unit-stride)
                tmq = tmps.tile([P, C * D], BF16, tag="tmp")
                pair_view = lambda ap: ap.rearrange(
                    "p (n dh two) -> p n dh two", two=2, dh=D // 2)
                ab = ad2[:, :, None, :].to_broadcast([P, w, D // 2, 2])
                nc.vector.tensor_tensor(
                    pair_view(tmq[:, 0:w * D]), pair_view(itb[:, 0:w * D]), ab, OP.mult)
                n_cur = w
                while n_cur > 2:
                    lo, hi = n_cur // 2, n_cur - n_cur // 2
                    nc.vector.tensor_tensor(
                        tmq[:, 0:lo * D], tmq[:, 0:lo * D],
                        tmq[:, hi * D:n_cur * D], OP.add)
                    n_cur = hi
                # final pair-add and accumulation run on GpSimd: acc isn't
                # needed until the normalize phase, so this is off the DVE
                # critical path entirely
                part = small.tile([P, D], F32, tag="part")
                nc.gpsimd.tensor_tensor(part[:], tmq[:, 0:D], tmq[:, D:2 * D], OP.add)
                nc.gpsimd.tensor_tensor(acc[:], acc[:], part[:], OP.add)

            # ---- normalize
            smax = small.tile([P, 1], F32, tag="sm")
            nc.vector.tensor_reduce(smax[:], s_all[:], axis=AX.X, op=OP.max)
            # sum of the duplicated buffer is exactly 2*denom
            denom = small.tile([P, 1], F32, tag="dn")
            nc.vector.tensor_reduce(denom[:], att_dup[:], axis=AX.X, op=OP.add)
            nc.vector.tensor_scalar(denom[:], denom[:], 0.5, None, OP.mult)
            # reference: if denom_shifted < 1e-7 then denom += exp(smax)
            es = small.tile([P, 1], F32, tag="es")
            nc.scalar.activation(es[:], smax[:], ACT.Exp)
            thr = small.tile([P, 1], F32, tag="th")
            nc.vector.tensor_scalar(thr[:], es[:], 1e-7, None, OP.mult)
            big = small.tile([P, 1], F32, tag="bg")
            nc.vector.scalar_tensor_tensor(
                big[:], denom[:], thr[:], es[:], op0=OP.is_lt, op1=OP.mult)
            nc.vector.tensor_tensor(denom[:], denom[:], big[:], OP.add)

            inv = small.tile([P, 1], F32, tag="iv")
            nc.vector.reciprocal(inv[:], denom[:])
            out_sb = work.tile([P, D], F32, tag="o")
            nc.vector.tensor_tensor(
                out_sb[:], acc[:], inv[:].to_broadcast([P, D]), OP.mult
            )
            nc.sync.dma_start(out[rows, :], out_sb[:])

    nc.compile()
    return nc


def _get_module() -> bass.Bass:
    if "nc" not in _CACHE:
        _CACHE["nc"] = build_module()
    return _CACHE["nc"]


def make_in_maps(item_embedding, query_embedding, mask, Wq, bq, Wr):
    import ml_dtypes

    bf16 = ml_dtypes.bfloat16
    item = np.ascontiguousarray(item_embedding, dtype=np.float32)
    qe = np.ascontiguousarray(query_embedding, dtype=np.float32)
    mk = np.ascontiguousarray(mask.reshape(B, N).astype(np.uint8))
    wq = np.ascontiguousarray(Wq, dtype=np.float32)
    bqr = np.ascontiguousarray(bq.reshape(1, DH).astype(bf16))
    wr = np.ascontiguousarray(Wr, dtype=np.float32)
    wr_rep = np.ascontiguousarray(np.tile(wr.reshape(1, H), (1, D)).astype(bf16))
    wqT = np.ascontiguousarray(wq.T.astype(bf16))
    in_maps = []
    for i in range(NCORES):
        r = slice(i * BS, (i + 1) * BS)
        in_maps.append({
            "item": item[r],
            "mask": mk[r],
            "bq": bqr,
            "Wr_rep": wr_rep,
            "WqT": wqT,
            "qeT": np.ascontiguousarray(qe[r].T.astype(bf16)),
        })
    return in_maps


def kernel(item_embedding, query_embedding, mask, Wq, bq, Wr):
    from concourse.bass_utils import run_bass_kernel_spmd

    nc = _get_module()
    in_maps = make_in_maps(item_embedding, query_embedding, mask, Wq, bq, Wr)
    last_err = None
    for attempt in range(3):
        try:
            res = run_bass_kernel_spmd(
                nc, in_maps, core_ids=list(range(NCORES)),
                **_CACHE.get("run_kwargs", {})
            )
            break
        except Exception as e:  # transient NRT_EXEC_UNIT_UNRECOVERABLE flakes
            last_err = e
    else:
        raise last_err
    _CACHE["last_results"] = res
    return np.concatenate([res.results[i]["out"] for i in range(NCORES)], axis=0)


# revision 84
# speedup vs baseline: 1.0567x; 1.0567x over previous
"""Trainium2 Bass kernel for nn_AttentionLayer (masked attention pooling).

Reference math (per batch row b):
    pq      = tanh(qe @ Wq.T + bq).reshape(D, H)
    dotted  = item_b @ pq                  # (N, H)
    reduced = dotted @ Wr.T                # (N, 1)
    ... masked softmax over N (max-subtracted, denom<1e-7 -> denom+1) ...
    out_b   = sum_n weight[n] * item_b[n]  # (D,)

Key algebraic simplification: reduced = item_b @ (pq @ Wr.T), so the H
dimension collapses into a per-row vector v_b = pq @ Wr.T of size D before
ever touching item_embedding.  Per row the work is then:
    scores = item_b @ v_b; masked softmax; out = weights @ item_b.

Distribution: pure data-parallel over batch across 8 cores (256 rows each).
Layout: batch rows on SBUF partitions (128 per tile), so every op is
per-partition independent -> VectorE/ScalarE; TensorE only does the shared
query projection qe @ Wq.T.

Softmax without max-subtraction: |score| <= ~50 so exp() stays within f32
range; softmax is shift-invariant so results match.  The reference's
`denom < 1e-7` test happens on max-subtracted values, reproduced here as
denom_raw < exp(smax + ln(1e-7)), and the +1 branch becomes
denom_raw + exp(smax).
"""

import sys

if "/opt/trn_rl_repo" not in sys.path:
    sys.path.insert(0, "/opt/trn_rl_repo")

from contextlib import ExitStack

import numpy as np

import concourse.bass as bass
import concourse.bacc as bacc
import concourse.tile as tile
from concourse import mybir

B, N, D, H = 2048, 200, 128, 8
NCORES = 8
BS = B // NCORES          # 256 batch rows per core
P = 128                   # partitions
NT = BS // P              # 2 batch tiles per core
C = 50                    # n-chunk size
NCH = N // C              # 4 chunks per batch tile
DH = D * H                # 1024

F32 = mybir.dt.float32
BF16 = mybir.dt.bfloat16
U8 = mybir.dt.uint8
AX = mybir.AxisListType
OP = mybir.AluOpType
ACT = mybir.ActivationFunctionType

_CACHE = {}


def build_module() -> bass.Bass:
    nc = bacc.Bacc("TRN2", target_bir_lowering=False)

    item = nc.declare_dram_parameter("item", [BS, N, D], F32, isOutput=False)
    mask = nc.declare_dram_parameter("mask", [BS, N], U8, isOutput=False)
    bq = nc.declare_dram_parameter("bq", [1, DH], BF16, isOutput=False)
    wr_rep_in = nc.declare_dram_parameter("Wr_rep", [1, DH], BF16, isOutput=False)
    wqT_in = nc.declare_dram_parameter("WqT", [D, DH], BF16, isOutput=False)
    qeT_in = nc.declare_dram_parameter("qeT", [D, BS], BF16, isOutput=False)
    out = nc.declare_dram_parameter("out", [BS, D], F32, isOutput=True)

    with tile.TileContext(nc) as tc, ExitStack() as ctx:
        const = ctx.enter_context(tc.tile_pool(name="const", bufs=1))
        psum = ctx.enter_context(tc.tile_pool(name="psum", bufs=2, space="PSUM"))
        work = ctx.enter_context(tc.tile_pool(name="work", bufs=2))
        items = ctx.enter_context(tc.tile_pool(name="items", bufs=3))
        tmps = ctx.enter_context(tc.tile_pool(name="tmps", bufs=3))
        small = ctx.enter_context(tc.tile_pool(name="small", bufs=4))

        # DMA issue order follows the dependency-chain lengths: wqT/qeT gate
        # the long PE->tanh->v chain, then the tiny bq/wr_rep transfers
        # (queues are FIFO — 2KB stuck behind an 800KB item quarter arrives
        # 8us late), then the first item chunk that gates the DVE stream.
        wqT = const.tile([P, DH], BF16)
        nc.sync.dma_start(wqT[:], wqT_in[:])
        qeT_all = const.tile([P, BS], BF16)
        nc.sync.dma_start(qeT_all[:], qeT_in[:])
        bq_sb = const.tile([1, DH], BF16)
        nc.sync.dma_start(bq_sb[:], bq[:])

        # Broadcast Wr (host-replicated to [1, D*H]) to all partitions with a
        # partition-stride-0 DMA read; the dense layout keeps the v-multiply
        # at full DVE rate (inner-broadcast APs run at a pathological rate).
        wr_rep = const.tile([P, DH], BF16)
        nc.sync.dma_start(wr_rep[:], wr_rep_in[0:1, :].to_broadcast([P, DH]))

        it0 = items.tile([P, C * D], F32, tag="item")
        src0 = item[0:P, 0:C, :].rearrange("b n d -> b (n d)")
        step0 = C * D // 4
        for q in range(4):
            nc.sync.dma_start(
                it0[:, q * step0:(q + 1) * step0], src0[:, q * step0:(q + 1) * step0])

        ones1 = const.tile([1, P], BF16)
        nc.vector.memset(ones1[:], 1.0)

        # ---- phase A for ALL tiles first, so ACT's static schedule finishes
        # the projections before starting the item converts that gate DVE.
        tiles = []
        for t in range(NT):
            rows = slice(t * P, (t + 1) * P)

            # query projection: v[b, d] = sum_h tanh(qe@WqT + bq) * Wr
            # (bf16 throughout: v is quantized to bf16 downstream anyway)
            pqt = work.tile([P, DH], BF16, tag="pqt")
            for j in range(2):
                js = slice(j * 512, (j + 1) * 512)
                pq_ps = psum.tile([P, 512], F32, tag="pq")
                nc.tensor.matmul(
                    pq_ps[:], qeT_all[:, rows], wqT[:, js], start=True, stop=False)
                nc.tensor.matmul(pq_ps[:], ones1[:], bq_sb[:, js], start=False, stop=True)
                nc.scalar.activation(pqt[:, js], pq_ps[:], ACT.Tanh)

            tmpv = work.tile([P, DH], BF16, tag="tmpv")
            tmpv3 = tmpv[:].rearrange("p (d h) -> p d h", h=H)
            nc.vector.tensor_tensor(tmpv[:], pqt[:], wr_rep[:], OP.mult)
            v_sb = work.tile([P, D], F32, tag="v")
            nc.vector.tensor_reduce(v_sb[:], tmpv3, axis=AX.X, op=OP.add)

            # bf16 working copies (all heavy DVE ops run bf16 at 2x)
            vb = work.tile([P, D], BF16, tag="vb")
            nc.vector.tensor_copy(out=vb[:], in_=v_sb[:])

            s_all = work.tile([P, N], F32, tag="s")
            # att is stored pair-duplicated ([a0,a0,a1,a1,...]) — written once
            # per chunk by a single fused multiply, consumed both by the
            # pair-packed wsum multiply and (summed, halved) by the denom
            att_dup = work.tile([P, 2 * N], BF16, tag="attd")
            acc = work.tile([P, D], F32, tag="acc")
            nc.gpsimd.memset(acc[:], 0.0)
            tiles.append((vb, s_all, att_dup, acc))

        for t in range(NT):
            rows = slice(t * P, (t + 1) * P)
            vb, s_all, att_dup, acc = tiles[t]

            # mask load sits here so its DMA descriptors don't delay the
            # first item chunks in the preamble queue (mask is first needed
            # only by the att multiply)
            mk_u8 = work.tile([P, N], U8, tag="mku")
            nc.sync.dma_start(mk_u8[:], mask[rows, :])
            mk_b = work.tile([P, N], BF16, tag="mkb")
            nc.vector.tensor_copy(out=mk_b[:], in_=mk_u8[:])

            # chunk list: (n-offset, width, f32 source AP or None).  Tile 0
            # splits its first chunk in two so compute starts as soon as the
            # first half of it0 lands.
            if t == 0:
                chunks = [(0, C // 2, it0[:, 0:C * D // 2]),
                          (C // 2, C // 2, it0[:, C * D // 2:C * D])]
                chunks += [(k * C, C, None) for k in range(1, NCH)]
            else:
                chunks = [(k * C, C, None) for k in range(NCH)]

            for ci, (off, w, it_src) in enumerate(chunks):
                ks = slice(off, off + w)
                if it_src is None:
                    it = items.tile([P, C * D], F32, tag="item")
                    # split the chunk DMA in halves to spread across queues
                    src = item[rows, ks, :].rearrange("b n d -> b (n d)")
                    hw = w * D // 2
                    nc.sync.dma_start(it[:, :hw], src[:, :hw])
                    nc.sync.dma_start(it[:, hw:w * D], src[:, hw:])
                    it_src = it[:, 0:w * D]

                # f32 -> bf16 convert on ACT (GpSimd CAST is ~4x slower and its
                # SBUF traffic stalls concurrent DVE ops).  The first chunks
                # convert on the still-idle DVE to shorten the ramp.
                itb = items.tile([P, C * D], BF16, tag="itb")
                if t == 0 and ci < 2:
                    nc.vector.tensor_copy(out=itb[:, 0:w * D], in_=it_src)
                else:
                    nc.scalar.copy(itb[:, 0:w * D], it_src)
                it3 = itb[:, 0:w * D].rearrange("p (n d) -> p n d", d=D)

                # scores chunk: s[p, n] = sum_d item * v
                # bf16 mult, dense halvings over d, then f32 reduce of 8
                tmp = tmps.tile([P, C * D], BF16, tag="tmp")
                tmp3 = tmp[:, 0:w * D].rearrange("p (n d) -> p n d", d=D)
                vb3 = vb[:, None, :].to_broadcast([P, w, D])
                nc.vector.tensor_tensor(tmp3, it3, vb3, OP.mult)
                dd = D
                while dd > 8:
                    dd //= 2
                    nc.vector.tensor_tensor(
                        tmp3[:, :, 0:dd], tmp3[:, :, 0:dd], tmp3[:, :, dd:2 * dd],
                        OP.add)
                nc.vector.tensor_reduce(
                    s_all[:, ks], tmp3[:, :, 0:8], axis=AX.X, op=OP.add)

                # att chunk: exp(s) * mask  (no max subtraction needed in f32)
                # att + pair-duplicate on DVE: tiny ops, and keeping them here
                # avoids two cross-engine semaphore hops in the chunk chain
                e_c = small.tile([P, C], BF16, tag="ec")
                nc.scalar.activation(e_c[:, 0:w], s_all[:, ks], ACT.Exp)
                # single fused op: compute e*mask directly into adjacent-pair
                # duplicated form (pairs pack into 32-bit reads -> 2x wmult)
                ad2 = att_dup[:, 2 * off:2 * (off + w)].rearrange(
                    "p (n two) -> p n two", two=2)
                nc.vector.tensor_tensor(
                    ad2,
                    e_c[:, 0:w][:, :, None].to_broadcast([P, w, 2]),
                    mk_b[:, ks][:, :, None].to_broadcast([P, w, 2]), OP.mult)

                # weighted accumulation: acc[p, d] += sum_n att * item
                # bf16 mult then dense pairwise tree over n (all unit-stride)
                tmq = tmps.tile([P, C * D], BF16, tag="tmp")
                pair_view = lambda ap: ap.rearrange(
                    "p (n dh two) -> p n dh two", two=2, dh=D // 2)
                ab = ad2[:, :, None, :].to_broadcast([P, w, D // 2, 2])
                nc.vector.tensor_tensor(
                    pair_view(tmq[:, 0:w * D]), pair_view(itb[:, 0:w * D]), ab, OP.mult)
                n_cur = w
                while n_cur > 2:
                    lo, hi = n_cur // 2, n_cur - n_cur // 2
                    nc.vector.tensor_tensor(
                        tmq[:, 0:lo * D], tmq[:, 0:lo * D],
                        tmq[:, hi * D:n_cur * D], OP.add)
                    n_cur = hi
                # final pair-add and accumulation run on GpSimd: acc isn't
                # needed until the normalize phase, so this is off the DVE
                # critical path entirely
                part = small.tile([P, D], F32, tag="part")
                nc.gpsimd.tensor_tensor(part[:], tmq[:, 0:D], tmq[:, D:2 * D], OP.add)
                nc.gpsimd.tensor_tensor(acc[:], acc[:], part[:], OP.add)

            # ---- normalize
            smax = small.tile([P, 1], F32, tag="sm")
            nc.vector.tensor_reduce(smax[:], s_all[:], axis=AX.X, op=OP.max)
            # sum of the duplicated buffer is exactly 2*denom
            denom = small.tile([P, 1], F32, tag="dn")
            nc.vector.tensor_reduce(denom[:], att_dup[:], axis=AX.X, op=OP.add)
            nc.vector.tensor_scalar(denom[:], denom[:], 0.5, None, OP.mult)
            # reference: if denom_shifted < 1e-7 then denom += exp(smax)
            es = small.tile([P, 1], F32, tag="es")
            nc.scalar.activation(es[:], smax[:], ACT.Exp)
            thr = small.tile([P, 1], F32, tag="th")
            nc.vector.tensor_scalar(thr[:], es[:], 1e-7, None, OP.mult)
            big = small.tile([P, 1], F32, tag="bg")
            nc.vector.scalar_tensor_tensor(
                big[:], denom[:], thr[:], es[:], op0=OP.is_lt, op1=OP.mult)
            nc.vector.tensor_tensor(denom[:], denom[:], big[:], OP.add)

            inv = small.tile([P, 1], F32, tag="iv")
            nc.vector.reciprocal(inv[:], denom[:])
            out_sb = work.tile([P, D], F32, tag="o")
            nc.vector.tensor_tensor(
                out_sb[:], acc[:], inv[:].to_broadcast([P, D]), OP.mult
            )
            nc.sync.dma_start(out[rows, :], out_sb[:])

    nc.compile()
    return nc


def _get_module() -> bass.Bass:
    if "nc" not in _CACHE:
        _CACHE["nc"] = build_module()
    return _CACHE["nc"]


def make_in_maps(item_embedding, query_embedding, mask, Wq, bq, Wr):
    import ml_dtypes

    bf16 = ml_dtypes.bfloat16
    item = np.ascontiguousarray(item_embedding, dtype=np.float32)
    qe = np.ascontiguousarray(query_embedding, dtype=np.float32)
    mk = np.ascontiguousarray(mask.reshape(B, N).astype(np.uint8))
    wq = np.ascontiguousarray(Wq, dtype=np.float32)
    bqr = np.ascontiguousarray(bq.reshape(1, DH).astype(bf16))
    wr = np.ascontiguousarray(Wr, dtype=np.float32)
    wr_rep = np.ascontiguousarray(np.tile(wr.reshape(1, H), (1, D)).astype(bf16))
    wqT = np.ascontiguousarray(wq.T.astype(bf16))
    in_maps = []
    for i in range(NCORES):
        r = slice(i * BS, (i + 1) * BS)
        in_maps.append({
            "item": item[r],
            "mask": mk[r],
            "bq": bqr,
            "Wr_rep": wr_rep,
            "WqT": wqT,
            "qeT": np.ascontiguousarray(qe[r].T.astype(bf16)),
        })
    return in_maps


def kernel(item_embedding, query_embedding, mask, Wq, bq, Wr):
    from concourse.bass_utils import run_bass_kernel_spmd

    nc = _get_module()
    in_maps = make_in_maps(item_embedding, query_embedding, mask, Wq, bq, Wr)
    last_err = None
    for attempt in range(3):
        try:
            res = run_bass_kernel_spmd(
                nc, in_maps, core_ids=list(range(NCORES)),
                **_CACHE.get("run_kwargs", {})
            )
            break
        except Exception as e:  # transient NRT_EXEC_UNIT_UNRECOVERABLE flakes
            last_err = e
    else:
        raise last_err
    _CACHE["last_results"] = res
    return np.concatenate([res.results[i]["out"] for i in range(NCORES)], axis=0)
